# revision 1
# baseline (speedup 1.0000x reference)
"""Trainium2 Bass kernel for nn_DihedralAngleLayer.

Input:  x [2_000_000, 42] f32 (14 atoms x 3 coords per row),
        mask_matrix [4, 14] f32 one-hot carbon selector.
Output: dihedral angle per row, [2_000_000] f32.

Data-parallel across 8 NeuronCores: rows are padded to 8*250_112 and split
evenly. Each core owns rows in global partition-major order: partition p
handles rows [p*Q, (p+1)*Q), Q = rows/128. Per tile (G columns of every
partition) the Vector engine computes

    a = c0-c1, b = c2-c1, d = c3-c2, m = b x d
    r=a.b p=a.d det=a.m q=b.b s=b.d       (dup-write + shifted-AP crosses)
    xx = p*q - r*s        # Lagrange identity for (a x b).(d x b)
    yy = sqrt(q)*det      # |v1| * scalar triple product

writing xx,yy into full-length planes; the per-row-scalar atan2 tail
(range-reduced arctan on ScalarE) runs on multi-tile chunks so the ~0.5us
per-instruction floor amortizes. dm2/da1 run on GPSIMD to offload DVE.
"""

import numpy as np

import concourse.bacc as bacc
import concourse.bass as bass
import concourse.mybir as mybir
from concourse.bass_utils import run_bass_kernel_spmd
from concourse.tile import TileContext

AF = mybir.ActivationFunctionType
OP = mybir.AluOpType
F32 = mybir.dt.float32

PI = float(np.pi)

N_CORES = 8
G_TILE = 192
# first two tiles halved so DVE starts ~4us sooner (smaller first DMA);
# trailing 34-tile gets its own chunk so almost no tail work is exposed
# after the last head finishes.
TILES = [96, 96] + [G_TILE] * 9 + [34]   # sum = 1954
CHUNK_AFTER = {3, 7, 10, 11}             # tile indices closing a tail chunk
Q = sum(TILES)                      # rows per partition
ROWS_PER_CORE = 128 * Q            # 250_112
TILES_PER_CHUNK = 4

# row-interleaved scratch layout per row-group (period 39 floats)
PER = 39
S_A, S_B2, S_D2, S_M = 0, 3, 9, 15
P_1, P_2 = 18, 21
D_0 = 24
# per-tile mini-planes ([G] each) for dots + pq/rs/sq: r,p,det,q,s,pq,rs,sq
RP_R, RP_P, RP_DET, RP_Q, RP_S, RP_PQ, RP_RS, RP_SQ = range(8)

# chunk-tail scratch: 8 slots of CS_FD floats each (slots reused over the chain)
CS_FD = G_TILE * TILES_PER_CHUNK


def _ap(base, off, dims):
    return bass.AP(
        base.tensor, base.offset + off, [list(base.ap[0])] + [list(d) for d in dims]
    )


def _emit_head(nc, xp, scp, rp, x, xyf, toff, G, c0, c1, c2, c3):
    """Per-tile head: subs, cross, dots, xx/yy -> full-length planes."""
    v, s, g = nc.vector, nc.scalar, nc.gpsimd

    xt = xp.tile([128, G * 42], F32, tag="x")
    sc = scp.tile([128, G * PER], F32, tag="sc")
    r5 = rp.tile([128, G * 8], F32, tag="r5")

    nc.gpsimd.dma_start(
        out=xt[:],
        in_=x.rearrange("(p q) c -> p q c", p=128)[:, toff : toff + G, :],
    )

    xa, sa, ra = xt[:], sc[:], r5[:]

    def xap(off, dims):
        return _ap(xa, off, [[42, G]] + dims)

    def sap(off, dims=()):
        return _ap(sa, off, [[PER, G]] + list(dims))

    def rap(k, dims=None):
        return _ap(ra, k * G, dims if dims is not None else [[1, G]])

    # a = c0-c1
    v.tensor_tensor(sap(S_A, [[1, 3]]), xap(c0, [[1, 3]]), xap(c1, [[1, 3]]), OP.subtract)
    # duplicated b = c2-c1 and d = c3-c2 (ISA allows max 3 free dims per AP)
    v.tensor_tensor(
        sap(S_B2, [[3, 2], [1, 3]]),
        xap(c2, [[0, 2], [1, 3]]),
        xap(c1, [[0, 2], [1, 3]]),
        OP.subtract,
    )
    v.tensor_tensor(
        sap(S_D2, [[3, 2], [1, 3]]),
        xap(c3, [[0, 2], [1, 3]]),
        xap(c2, [[0, 2], [1, 3]]),
        OP.subtract,
    )
    # P1 = b_yzx*d_zxy ; P2 = b_zxy*d_yzx
    v.tensor_tensor(
        sap(P_1, [[3, 2], [1, 3]]),
        sap(S_B2 + 1, [[1, 2], [1, 3]]),
        sap(S_D2 + 2, [[-1, 2], [1, 3]]),
        OP.mult,
    )
    # m = P1 - P2
    v.tensor_tensor(sap(S_M, [[1, 3]]), sap(P_1, [[1, 3]]), sap(P_2, [[1, 3]]), OP.subtract)
    # three-prods of a with {b,d,m} -> rt,pt,dett   (DVE)
    v.tensor_tensor(
        sap(D_0, [[3, 3], [1, 3]]),
        sap(S_A, [[0, 3], [1, 3]]),
        sap(S_B2, [[6, 3], [1, 3]]),
        OP.mult,
    )
    # qt = b*b on ScalarE (Square is a filler in every ACT table set, and the
    # Scalar engine is far from saturated); st = b*d stays on DVE.
    # (GPSIMD tensor work is a net loss: it contends with DVE on the shared
    #  SBUF port and inflates every concurrent DVE op up to 2.4x — measured.)
    s.activation(sap(D_0 + 9, [[1, 3]]), sap(S_B2, [[1, 3]]), AF.Square)
    v.tensor_tensor(
        sap(D_0 + 12, [[1, 3]]),
        sap(S_B2, [[1, 3]]),
        sap(S_D2, [[1, 3]]),
        OP.mult,
    )
    # segmented reduce -> dots r,p,det,q,s as per-tile mini-planes: iterating
    # (dot, row, comp) makes both the reads and the plane writes unit-inner
    v.reduce_sum(
        rap(RP_R, [[G, 5], [1, G]]),
        _ap(sa, D_0, [[3, 5], [PER, G], [1, 3]]),
        axis=mybir.AxisListType.X,
    )
    # [pq, rs] on planes
    v.tensor_tensor(
        rap(RP_PQ, [[G, 2], [1, G]]),
        rap(RP_P, [[-G, 2], [1, G]]),
        rap(RP_Q, [[G, 2], [1, G]]),
        OP.mult,
    )
    # xx -> full plane (all unit stride)
    v.tensor_tensor(_ap(xyf, toff, [[1, G]]), rap(RP_PQ), rap(RP_RS), OP.subtract)
    # sq = sqrt(q); yy = sq*det -> full plane
    s.activation(rap(RP_SQ), rap(RP_Q), AF.Sqrt)
    v.tensor_tensor(_ap(xyf, Q + toff, [[1, G]]), rap(RP_SQ), rap(RP_DET), OP.mult)


def _emit_tail(nc, csp, outp, y, xyf, toff, FD):
    """Chunk tail: atan2 on [128, FD] contiguous planes."""
    v, s = nc.vector, nc.scalar

    cs = csp.tile([128, 7 * CS_FD], F32, tag="cs")
    ot = outp.tile([128, CS_FD], F32, tag="o")
    ca = cs[:]

    def cap(k, n=1):
        return _ap(ca, k * CS_FD, [[1, FD]] if n == 1 else [[CS_FD, n], [1, FD]])

    def xy(n=1):
        return _ap(xyf, toff, [[Q, n], [1, FD]] if n > 1 else [[1, FD]])

    # slots: 0:ax/e2  1:ay/sy  2:df/u  3:mn/v  4:mx/rq  5:rmx/al  6:e1/z
    s.activation(cap(0, 2), xy(2), AF.Abs)                       # ax,ay
    v.tensor_tensor(cap(2), cap(0), cap(1), OP.subtract)          # df
    v.tensor_tensor(cap(3), cap(0), cap(1), OP.min)               # mn
    v.tensor_tensor(cap(4), cap(0), cap(1), OP.max)               # mx
    v.reciprocal_approx_fast(cap(5), cap(4))                      # rmx
    v.tensor_tensor(cap(4), cap(3), cap(5), OP.mult)              # rq (mx slot)
    s.activation(cap(5), cap(4), AF.Arctan)                       # al (rmx slot)
    s.activation(cap(6), cap(2), AF.Sign)                         # e1
    s.activation(cap(0, 2), xy(2), AF.Sign)                       # e2,sy (ax/ay slots)
    v.tensor_tensor(cap(2), cap(6), cap(0), OP.mult)              # u (df slot)
    v.tensor_tensor(cap(3), cap(5), cap(2), OP.mult)              # v (mn slot)
    v.scalar_tensor_tensor(cap(5), cap(2), PI / 4, cap(3), OP.mult, OP.subtract)  # w2
    v.scalar_tensor_tensor(cap(6), cap(0), PI / 4, cap(5), OP.mult, OP.add)       # z
    v.scalar_tensor_tensor(
        _ap(ot[:], 0, [[1, FD]]), cap(6), PI / 2, cap(1), OP.subtract, OP.mult
    )
    nc.gpsimd.dma_start(
        out=y.rearrange("(p q) -> p q", p=128)[:, toff : toff + FD],
        in_=_ap(ot[:], 0, [[1, FD]]),
    )


def build_kernel(atoms):
    c0, c1, c2, c3 = (3 * int(a) for a in atoms)
    nc = bacc.Bacc("TRN2", target_bir_lowering=False, debug=False)
    x = nc.dram_tensor("x", [ROWS_PER_CORE, 42], F32, kind="ExternalInput")
    y = nc.dram_tensor("y", [ROWS_PER_CORE], F32, kind="ExternalOutput")
    with TileContext(nc) as tc:
        with (
            tc.tile_pool(name="xp", bufs=2) as xp,
            tc.tile_pool(name="scp", bufs=2) as scp,
            tc.tile_pool(name="rp", bufs=2) as rp,
            tc.tile_pool(name="xyp", bufs=1) as xyp,
            tc.tile_pool(name="csp", bufs=1) as csp,
            tc.tile_pool(name="outp", bufs=2) as outp,
        ):
            xyf_tile = xyp.tile([128, 2 * Q], F32, tag="xy")
            xyf = xyf_tile[:]
            toff = 0
            chunk_start = 0
            for i, G in enumerate(TILES):
                _emit_head(nc, xp, scp, rp, x, xyf, toff, G, c0, c1, c2, c3)
                toff += G
                if i in CHUNK_AFTER or i == len(TILES) - 1:
                    _emit_tail(nc, csp, outp, y, xyf, chunk_start, toff - chunk_start)
                    chunk_start = toff
    nc.finalize()
    return nc


_CACHE = {}


def _get_nc(atoms):
    key = tuple(int(a) for a in atoms)
    if key not in _CACHE:
        _CACHE[key] = build_kernel(key)
    return _CACHE[key]


def run(x, atoms=(0, 4, 7, 11), **spmd_kwargs):
    """x: [B, 42] f32. Returns (y [B] f32, BassKernelResults)."""
    x = np.ascontiguousarray(np.asarray(x, dtype=np.float32))
    B = x.shape[0]
    total = N_CORES * ROWS_PER_CORE
    if B < total:
        # pad with replicated leading rows (valid, non-degenerate data)
        x = np.concatenate([x, x[: total - B]], axis=0)
    nc = _get_nc(atoms)
    shards = x.reshape(N_CORES, ROWS_PER_CORE, 42)
    in_maps = [{"x": shards[i]} for i in range(N_CORES)]
    res = run_bass_kernel_spmd(nc, in_maps, core_ids=list(range(N_CORES)), **spmd_kwargs)
    y = np.concatenate([r["y"] for r in res.results])[:B]
    return np.asarray(y, dtype=np.float32), res


def kernel(x, mask_matrix):
    mask = np.asarray(mask_matrix)
    atoms = tuple(int(i) for i in np.argmax(mask, axis=1))
    y, _ = run(x, atoms=atoms)
    return y



# revision 2
# speedup vs baseline: 1.0529x; 1.0529x over previous
"""Trainium2 Bass kernel for nn_DihedralAngleLayer.

Input:  x [2_000_000, 42] f32 (14 atoms x 3 coords per row),
        mask_matrix [4, 14] f32 one-hot carbon selector.
Output: dihedral angle per row, [2_000_000] f32.

Data-parallel across 8 NeuronCores: rows are padded to 8*250_112 and split
evenly. Each core owns rows in global partition-major order: partition p
handles rows [p*Q, (p+1)*Q), Q = rows/128. Per tile (G columns of every
partition) the Vector engine computes

    a = c0-c1, b = c2-c1, d = c3-c2, m = b x d
    r=a.b p=a.d det=a.m s=b.d q=b.b
    xx = p*q - r*s        # Lagrange identity for (a x b).(d x b)
    yy = sqrt(q)*det      # |v1| * scalar triple product

DVE-lean layout: the b/d duplicates needed by the shifted-AP cross reads
are written by ScalarE Copy (not dup-write TTs); the five dots are folded
by two shifted-AP adds (10 elems/row) instead of TENSOR_REDUCE (15); the
atan2 tail uses the full-range ACT Arctan table (verified 5e-7 max err
over all f32 including +-inf):  t = arctan(yy/|xx|), then a 3-op quadrant
fix via e2=sign(xx), sy=sign(yy).  DVE work: 44/row head + 5/row tail
(was 55 + 10), the rest rides on ScalarE.
"""

import numpy as np

import concourse.bacc as bacc
import concourse.bass as bass
import concourse.mybir as mybir
from concourse.bass_utils import run_bass_kernel_spmd
from concourse.tile import TileContext

AF = mybir.ActivationFunctionType
OP = mybir.AluOpType
F32 = mybir.dt.float32

PI = float(np.pi)

N_CORES = 8
G_TILE = 192
# first two tiles halved so DVE starts ~4us sooner (smaller first DMA);
# trailing 34-tile gets its own chunk so almost no tail work is exposed
# after the last head finishes.
TILES = [96, 96] + [G_TILE] * 9 + [34]   # sum = 1954
CHUNK_AFTER = {3, 7, 10, 11}             # tile indices closing a tail chunk
Q = sum(TILES)                      # rows per partition
ROWS_PER_CORE = 128 * Q            # 250_112
TILES_PER_CHUNK = 4

# row-interleaved scratch layout per row-group (period PER floats)
PER = 33
S_A, S_B, S_D, S_M, PROD = 0, 3, 9, 15, 18
# r5 mini-planes ([G] each): t1 x5, dots r,p,det,s,q (5..9), pq(10), rs(11), sq(12)
N_PLANES = 13

# chunk-tail scratch: 5 slots of CS_FD floats each
CS_FD = G_TILE * TILES_PER_CHUNK


def _ap(base, off, dims):
    return bass.AP(
        base.tensor, base.offset + off, [list(base.ap[0])] + [list(d) for d in dims]
    )


def _emit_head(nc, xp, scp, rp, x, xyf, toff, G, c0, c1, c2, c3):
    """Per-tile head: subs, cross, dots, xx/yy -> full-length planes."""
    v, s = nc.vector, nc.scalar

    xt = xp.tile([128, G * 42], F32, tag="x")
    sc = scp.tile([128, G * PER], F32, tag="sc")
    r5 = rp.tile([128, G * N_PLANES], F32, tag="r5")

    nc.gpsimd.dma_start(
        out=xt[:],
        in_=x.rearrange("(p q) c -> p q c", p=128)[:, toff : toff + G, :],
    )

    xa, sa, ra = xt[:], sc[:], r5[:]

    def xap(off, dims):
        return _ap(xa, off, [[42, G]] + dims)

    def sap(off, dims=()):
        return _ap(sa, off, [[PER, G]] + list(dims))

    def pl(k, dims=None):
        return _ap(ra, k * G, dims if dims is not None else [[1, G]])

    # a = c0-c1
    v.tensor_tensor(sap(S_A, [[1, 3]]), xap(c0, [[1, 3]]), xap(c1, [[1, 3]]), OP.subtract)
    # (b, d) = (c2, c3) - (c1, c2) in one TT, single copies
    v.tensor_tensor(
        sap(S_B, [[6, 2], [1, 3]]),
        xap(c2, [[12, 2], [1, 3]]),
        xap(c1, [[9, 2], [1, 3]]),
        OP.subtract,
    )
    # duplicates for the shifted-AP cross reads, off-DVE on ScalarE
    s.activation(sap(S_B + 3, [[6, 2], [1, 3]]), sap(S_B, [[6, 2], [1, 3]]), AF.Copy)
    # P1 = b_yzx*d_zxy ; P2 = b_zxy*d_yzx  (aliased over PROD, consumed by m)
    v.tensor_tensor(
        sap(PROD, [[3, 2], [1, 3]]),
        sap(S_B + 1, [[1, 2], [1, 3]]),
        sap(S_D + 2, [[-1, 2], [1, 3]]),
        OP.mult,
    )
    # m = P1 - P2
    v.tensor_tensor(sap(S_M, [[1, 3]]), sap(PROD, [[1, 3]]), sap(PROD + 3, [[1, 3]]), OP.subtract)
    # (ab, ad, am) products -> PROD..PROD+9
    v.tensor_tensor(
        sap(PROD, [[3, 3], [1, 3]]),
        sap(S_A, [[0, 3], [1, 3]]),
        sap(S_B, [[6, 3], [1, 3]]),
        OP.mult,
    )
    # s = b*d -> PROD+9
    v.tensor_tensor(sap(PROD + 9, [[1, 3]]), sap(S_B, [[1, 3]]), sap(S_D, [[1, 3]]), OP.mult)
    # q = b*b on ScalarE -> PROD+12
    s.activation(sap(PROD + 12, [[1, 3]]), sap(S_B, [[1, 3]]), AF.Square)
    # dots via two shifted adds: t1 = P[.,0]+P[.,1] -> planes 0..4,
    # dots = t1 + P[.,2] -> planes 5..9 (r,p,det,s,q)
    v.tensor_tensor(pl(0, [[1, G], [G, 5]]), sap(PROD, [[3, 5]]), sap(PROD + 1, [[3, 5]]), OP.add)
    v.tensor_tensor(
        pl(5, [[1, G], [G, 5]]), pl(0, [[1, G], [G, 5]]), sap(PROD + 2, [[3, 5]]), OP.add
    )
    # (pq, rs) = (p,r)*(q,s) -> planes 10,11
    v.tensor_tensor(
        pl(10, [[1, G], [G, 2]]), pl(6, [[1, G], [-G, 2]]), pl(9, [[1, G], [-G, 2]]), OP.mult
    )
    # xx -> full plane
    v.tensor_tensor(_ap(xyf, toff, [[1, G]]), pl(10), pl(11), OP.subtract)
    # sq = sqrt(q); yy = sq*det -> full plane
    s.activation(pl(12), pl(9), AF.Sqrt)
    v.tensor_tensor(_ap(xyf, Q + toff, [[1, G]]), pl(12), pl(7), OP.mult)


def _emit_tail(nc, csp, outp, y, xyf, toff, FD):
    """Chunk tail: atan2 on [128, FD] contiguous planes via full-range arctan.

    t = arctan(yy/|xx|); theta = e2*t + (pi/2)*(1-e2)*sy with e2=sign(xx),
    sy=sign(yy).  (STT subtract is reversed: out = in1 - (in0 op0 scalar).)
    """
    v, s = nc.vector, nc.scalar

    cs = csp.tile([128, 5 * CS_FD], F32, tag="cs")
    ot = outp.tile([128, CS_FD], F32, tag="o")
    ca = cs[:]

    def cap(k):
        return _ap(ca, k * CS_FD, [[1, FD]])

    def xy(off=0):
        return _ap(xyf, toff + off, [[1, FD]])

    # slots: 0:ax/w  1:rx/v2  2:rq/t->2 stays t  3:e2  4:sy
    s.activation(cap(0), xy(0), AF.Abs)                            # ax = |xx|
    v.reciprocal_approx_fast(cap(1), cap(0))                       # rx
    v.tensor_tensor(cap(2), xy(Q), cap(1), OP.mult)                # rq = yy*rx
    s.activation(cap(2), cap(2), AF.Arctan)                        # t (in place)
    s.activation(
        _ap(ca, 3 * CS_FD, [[CS_FD, 2], [1, FD]]),
        _ap(xyf, toff, [[Q, 2], [1, FD]]),
        AF.Sign,
    )                                                              # e2, sy
    v.scalar_tensor_tensor(cap(0), cap(3), 1.0, cap(4), OP.subtract, OP.mult)  # w=(e2-1)*sy
    v.tensor_tensor(cap(1), cap(3), cap(2), OP.mult)               # v2 = e2*t
    # out = v2 - w*pi/2  (reversed subtract)
    v.scalar_tensor_tensor(
        _ap(ot[:], 0, [[1, FD]]), cap(0), PI / 2, cap(1), OP.mult, OP.subtract
    )
    nc.gpsimd.dma_start(
        out=y.rearrange("(p q) -> p q", p=128)[:, toff : toff + FD],
        in_=_ap(ot[:], 0, [[1, FD]]),
    )


def build_kernel(atoms):
    c0, c1, c2, c3 = (3 * int(a) for a in atoms)
    nc = bacc.Bacc("TRN2", target_bir_lowering=False, debug=False)
    x = nc.dram_tensor("x", [ROWS_PER_CORE, 42], F32, kind="ExternalInput")
    y = nc.dram_tensor("y", [ROWS_PER_CORE], F32, kind="ExternalOutput")
    with TileContext(nc) as tc:
        with (
            tc.tile_pool(name="xp", bufs=2) as xp,
            tc.tile_pool(name="scp", bufs=2) as scp,
            tc.tile_pool(name="rp", bufs=2) as rp,
            tc.tile_pool(name="xyp", bufs=1) as xyp,
            tc.tile_pool(name="csp", bufs=1) as csp,
            tc.tile_pool(name="outp", bufs=2) as outp,
        ):
            xyf_tile = xyp.tile([128, 2 * Q], F32, tag="xy")
            xyf = xyf_tile[:]
            toff = 0
            chunk_start = 0
            for i, G in enumerate(TILES):
                _emit_head(nc, xp, scp, rp, x, xyf, toff, G, c0, c1, c2, c3)
                toff += G
                if i in CHUNK_AFTER or i == len(TILES) - 1:
                    _emit_tail(nc, csp, outp, y, xyf, chunk_start, toff - chunk_start)
                    chunk_start = toff
    nc.finalize()
    return nc


_CACHE = {}


def _get_nc(atoms):
    key = tuple(int(a) for a in atoms)
    if key not in _CACHE:
        _CACHE[key] = build_kernel(key)
    return _CACHE[key]


def run(x, atoms=(0, 4, 7, 11), **spmd_kwargs):
    """x: [B, 42] f32. Returns (y [B] f32, BassKernelResults)."""
    x = np.ascontiguousarray(np.asarray(x, dtype=np.float32))
    B = x.shape[0]
    total = N_CORES * ROWS_PER_CORE
    if B < total:
        # pad with replicated leading rows (valid, non-degenerate data)
        x = np.concatenate([x, x[: total - B]], axis=0)
    nc = _get_nc(atoms)
    shards = x.reshape(N_CORES, ROWS_PER_CORE, 42)
    in_maps = [{"x": shards[i]} for i in range(N_CORES)]
    res = run_bass_kernel_spmd(nc, in_maps, core_ids=list(range(N_CORES)), **spmd_kwargs)
    y = np.concatenate([r["y"] for r in res.results])[:B]
    return np.asarray(y, dtype=np.float32), res


def kernel(x, mask_matrix):
    mask = np.asarray(mask_matrix)
    atoms = tuple(int(i) for i in np.argmax(mask, axis=1))
    y, _ = run(x, atoms=atoms)
    return y


# revision 3
# speedup vs baseline: 1.0788x; 1.0246x over previous
"""Trainium2 Bass kernel for nn_DihedralAngleLayer.

Input:  x [2_000_000, 42] f32 (14 atoms x 3 coords per row),
        mask_matrix [4, 14] f32 one-hot carbon selector.
Output: dihedral angle per row, [2_000_000] f32.

Data-parallel across 8 NeuronCores: rows are padded to 8*250_112 and split
evenly. Each core owns rows in global partition-major order: partition p
handles rows [p*Q, (p+1)*Q), Q = rows/128. Per tile (G columns of every
partition) the Vector engine computes

    a = c0-c1, b = c2-c1, d = c3-c2
    na = a x b, nb = d x b
    det = na.d, xx = na.nb, q = b.b      (one 9-read segmented reduce
                                          straight into full planes)
    yy = sqrt(q)*det

The x/y duplicates the shifted-AP cross reads need are written by ScalarE
Copy (only 2 dup elems per vector are ever read); q = b*b runs on ScalarE
Square. The atan2 tail uses the full-range ACT Arctan table (verified
5e-7 max err over all f32 incl +-inf): t = arctan(yy/|xx|) plus a 3-op
quadrant fix from e2=sign(xx), sy=sign(yy). DVE work: ~43 elems/row head
+ 5/row tail (baseline was 55 + 10).
"""

import numpy as np

import concourse.bacc as bacc
import concourse.bass as bass
import concourse.mybir as mybir
from concourse.bass_utils import run_bass_kernel_spmd
from concourse.tile import TileContext

AF = mybir.ActivationFunctionType
OP = mybir.AluOpType
F32 = mybir.dt.float32

PI = float(np.pi)

N_CORES = 8
G_TILE = 192
# first two tiles halved so DVE starts ~4us sooner (smaller first DMA);
# trailing 34-tile gets its own chunk so almost no tail work is exposed
# after the last head finishes.
TILES = [96, 96] + [G_TILE] * 9 + [34]   # sum = 1954
CHUNK_AFTER = {3, 7, 10, 11}             # tile indices closing a tail chunk
Q = sum(TILES)                      # rows per partition
ROWS_PER_CORE = 128 * Q            # 250_112
TILES_PER_CHUNK = 4

# row-interleaved scratch layout per row-group (period PER floats):
# a,a'(0..6) b,b'(6..12) d,d'(12..18) na(18) nb(21) P12(24,27 transient)
# prods: na.d(24) na.nb(27) b.b(30)  -> PER=33
PER = 33
S_A, S_B, S_D, S_NA, S_NB, S_P = 0, 6, 12, 18, 21, 24

# chunk-tail scratch: 5 slots of CS_FD floats each
CS_FD = G_TILE * TILES_PER_CHUNK


def _ap(base, off, dims):
    return bass.AP(
        base.tensor, base.offset + off, [list(base.ap[0])] + [list(d) for d in dims]
    )


def _emit_head(nc, xp, scp, rp, x, xyf, toff, G, c0, c1, c2, c3):
    """Per-tile head: subs, crosses, fused dot-reduce -> full-length planes."""
    v, s = nc.vector, nc.scalar

    xt = xp.tile([128, G * 42], F32, tag="x")
    sc = scp.tile([128, G * PER], F32, tag="sc")
    r5 = rp.tile([128, G], F32, tag="r5")

    nc.gpsimd.dma_start(
        out=xt[:],
        in_=x.rearrange("(p q) c -> p q c", p=128)[:, toff : toff + G, :],
    )

    xa, sa = xt[:], sc[:]

    def xap(off, dims):
        return _ap(xa, off, [[42, G]] + dims)

    def sap(off, dims=()):
        return _ap(sa, off, [[PER, G]] + list(dims))

    # a = c0-c1
    v.tensor_tensor(sap(S_A, [[1, 3]]), xap(c0, [[1, 3]]), xap(c1, [[1, 3]]), OP.subtract)
    # (b, d) = (c2, c3) - (c1, c2) in one TT
    v.tensor_tensor(
        sap(S_B, [[6, 2], [1, 3]]),
        xap(c2, [[12, 2], [1, 3]]),
        xap(c1, [[9, 2], [1, 3]]),
        OP.subtract,
    )
    # x/y duplicates for the shifted-AP cross reads (z dup is never read),
    # off-DVE on ScalarE
    s.activation(sap(S_A + 3, [[6, 3], [1, 2]]), sap(S_A, [[6, 3], [1, 2]]), AF.Copy)
    # na = a x b : P1 = a_yzx*b_zxy ; P2 = a_zxy*b_yzx
    v.tensor_tensor(
        sap(S_P, [[3, 2], [1, 3]]),
        sap(S_A + 1, [[1, 2], [1, 3]]),
        sap(S_B + 2, [[-1, 2], [1, 3]]),
        OP.mult,
    )
    v.tensor_tensor(sap(S_NA, [[1, 3]]), sap(S_P, [[1, 3]]), sap(S_P + 3, [[1, 3]]), OP.subtract)
    # nb = d x b : P1' = d_yzx*b_zxy ; P2' = d_zxy*b_yzx
    v.tensor_tensor(
        sap(S_P, [[3, 2], [1, 3]]),
        sap(S_D + 1, [[1, 2], [1, 3]]),
        sap(S_B + 2, [[-1, 2], [1, 3]]),
        OP.mult,
    )
    v.tensor_tensor(sap(S_NB, [[1, 3]]), sap(S_P, [[1, 3]]), sap(S_P + 3, [[1, 3]]), OP.subtract)
    # products (na*d, na*nb) -> P..P+6
    v.tensor_tensor(
        sap(S_P, [[3, 2], [1, 3]]),
        sap(S_NA, [[0, 2], [1, 3]]),
        sap(S_D, [[9, 2], [1, 3]]),
        OP.mult,
    )
    # b*b on ScalarE -> P+6
    s.activation(sap(S_P + 6, [[1, 3]]), sap(S_B, [[1, 3]]), AF.Square)
    # segmented reduce (det, xx, q) straight into the full planes:
    # det -> xyf[toff], xx -> xyf[Q+toff], q -> xyf[2Q+toff]
    v.reduce_sum(
        _ap(xyf, toff, [[Q, 3], [1, G]]),
        _ap(sa, S_P, [[3, 3], [PER, G], [1, 3]]),
        axis=mybir.AxisListType.X,
    )
    # sq = sqrt(q); yy = sq*det overwrites the det plane
    s.activation(_ap(r5[:], 0, [[1, G]]), _ap(xyf, 2 * Q + toff, [[1, G]]), AF.Sqrt)
    v.tensor_tensor(
        _ap(xyf, toff, [[1, G]]), _ap(xyf, toff, [[1, G]]), _ap(r5[:], 0, [[1, G]]), OP.mult
    )


def _emit_tail(nc, csp, outp, y, xyf, toff, FD):
    """Chunk tail: atan2 on [128, FD] contiguous planes via full-range arctan.

    t = arctan(yy/|xx|); theta = e2*t + (pi/2)*(1-e2)*sy with e2=sign(xx),
    sy=sign(yy).  (STT subtract is reversed: out = in1 - (in0 op0 scalar).)
    """
    v, s = nc.vector, nc.scalar

    cs = csp.tile([128, 5 * CS_FD], F32, tag="cs")
    ot = outp.tile([128, CS_FD], F32, tag="o")
    ca = cs[:]

    def cap(k):
        return _ap(ca, k * CS_FD, [[1, FD]])

    # planes: yy @ toff, xx @ Q+toff
    # slots: 0:ax/w  1:rx/v2  2:rq/t  3:e2  4:sy
    s.activation(cap(0), _ap(xyf, Q + toff, [[1, FD]]), AF.Abs)    # ax = |xx|
    v.reciprocal_approx_fast(cap(1), cap(0))                       # rx
    v.tensor_tensor(cap(2), _ap(xyf, toff, [[1, FD]]), cap(1), OP.mult)  # rq = yy*rx
    s.activation(cap(2), cap(2), AF.Arctan)                        # t (in place)
    s.activation(
        _ap(ca, 3 * CS_FD, [[CS_FD, 2], [1, FD]]),
        _ap(xyf, Q + toff, [[-Q, 2], [1, FD]]),
        AF.Sign,
    )                                                              # e2, sy
    v.scalar_tensor_tensor(cap(0), cap(3), 1.0, cap(4), OP.subtract, OP.mult)  # w=(e2-1)*sy
    v.tensor_tensor(cap(1), cap(3), cap(2), OP.mult)               # v2 = e2*t
    # out = v2 - w*pi/2  (reversed subtract)
    v.scalar_tensor_tensor(
        _ap(ot[:], 0, [[1, FD]]), cap(0), PI / 2, cap(1), OP.mult, OP.subtract
    )
    nc.gpsimd.dma_start(
        out=y.rearrange("(p q) -> p q", p=128)[:, toff : toff + FD],
        in_=_ap(ot[:], 0, [[1, FD]]),
    )


def build_kernel(atoms):
    c0, c1, c2, c3 = (3 * int(a) for a in atoms)
    nc = bacc.Bacc("TRN2", target_bir_lowering=False, debug=False)
    x = nc.dram_tensor("x", [ROWS_PER_CORE, 42], F32, kind="ExternalInput")
    y = nc.dram_tensor("y", [ROWS_PER_CORE], F32, kind="ExternalOutput")
    with TileContext(nc) as tc:
        with (
            tc.tile_pool(name="xp", bufs=2) as xp,
            tc.tile_pool(name="scp", bufs=2) as scp,
            tc.tile_pool(name="rp", bufs=2) as rp,
            tc.tile_pool(name="xyp", bufs=1) as xyp,
            tc.tile_pool(name="csp", bufs=1) as csp,
            tc.tile_pool(name="outp", bufs=2) as outp,
        ):
            xyf_tile = xyp.tile([128, 3 * Q], F32, tag="xy")
            xyf = xyf_tile[:]
            toff = 0
            chunk_start = 0
            for i, G in enumerate(TILES):
                _emit_head(nc, xp, scp, rp, x, xyf, toff, G, c0, c1, c2, c3)
                toff += G
                if i in CHUNK_AFTER or i == len(TILES) - 1:
                    _emit_tail(nc, csp, outp, y, xyf, chunk_start, toff - chunk_start)
                    chunk_start = toff
    nc.finalize()
    return nc


_CACHE = {}


def _get_nc(atoms):
    key = tuple(int(a) for a in atoms)
    if key not in _CACHE:
        _CACHE[key] = build_kernel(key)
    return _CACHE[key]


def run(x, atoms=(0, 4, 7, 11), **spmd_kwargs):
    """x: [B, 42] f32. Returns (y [B] f32, BassKernelResults)."""
    x = np.ascontiguousarray(np.asarray(x, dtype=np.float32))
    B = x.shape[0]
    total = N_CORES * ROWS_PER_CORE
    if B < total:
        # pad with replicated leading rows (valid, non-degenerate data)
        x = np.concatenate([x, x[: total - B]], axis=0)
    nc = _get_nc(atoms)
    shards = x.reshape(N_CORES, ROWS_PER_CORE, 42)
    in_maps = [{"x": shards[i]} for i in range(N_CORES)]
    res = run_bass_kernel_spmd(nc, in_maps, core_ids=list(range(N_CORES)), **spmd_kwargs)
    y = np.concatenate([r["y"] for r in res.results])[:B]
    return np.asarray(y, dtype=np.float32), res


def kernel(x, mask_matrix):
    mask = np.asarray(mask_matrix)
    atoms = tuple(int(i) for i in np.argmax(mask, axis=1))
    y, _ = run(x, atoms=atoms)
    return y
